# revision 25
# baseline (speedup 1.0000x reference)
"""Causal multi-head attention with (buggy-faithful) RoPE on 8 trn2 cores.

Problem: B=4, S=2048, D=1024, H=16 heads of dim 64, fp32.
Sharding: core c handles batch b=c//2 and head-group g=c%2 (8 heads).
Each core computes partial_out = attn(x_b, heads g) @ wo[rows g]; the host
sums the two partials per batch and adds the bias.

Key host-side preprocessing:
- The reference's RoPE (with its cos-overwritten-by-sin bug) reduces to
  q' = (q @ (I + R')) * sin_pattern, where R' swaps/negates half-dims.
  (I + R') is folded into wq/wk on the host, so on-device RoPE is a single
  elementwise multiply fused into the PSUM->SBUF drain of the projections.
- Q/K projections run as fp8 DoubleRow matmuls (two 128-deep k-chunks per
  instruction): x is shipped twice (fp8 chunk-paired for Q/K, bf16 for V),
  wq/wk are fp8 scaled by 8 (mid e4m3 range; the 8*8=64 score scale folds
  into the exp scale 0.125/64 = 1/512). The V path stays bf16 end-to-end:
  fp8 V errors do not average out at early seq positions (few valid keys)
  and blow the max-error budget.

On-device layout (per core):
- QT/KT [512, 2048] bf16 feature-major (head pairs per 128-partition chunk)
- V [128, 2, 8, 65] bf16 per seq chunk-pair, 65 cols per head (64 + ones
  col -> softmax denominator accumulates for free in the P@V matmul)
- S^T [sk, sq] per head in a [128, 2(kc), 2(e), 512] psum batch (2 k-chunks
  x 2 parity heads); ONE exp per batch (fewer ACT fixed overheads); causal
  mask via affine_select on the bf16 pt tile. The parity score matmuls use
  K=64 row-groups 0-1/2-3 so they pack concurrently on the PE array.
- P@V accumulates per chunk, restricted to the causally-needed column
  range on diagonal chunks; runs one chunk-pair behind the scores so PE
  never waits on ACT. exp needs no max subtraction (|scores/8| < ~3).
"""

import numpy as np
import ml_dtypes

import concourse.bacc as bacc
import concourse.mybir as mybir
import concourse.tile as tile
from concourse.bass_utils import run_bass_kernel_spmd

B, S, D = 4, 2048, 1024
H = 16
AOD = 64
HL = 8            # heads per core
FL = HL * AOD     # 512 local features
NCORES = 8
NKP = D // 256    # 4 contraction chunk-pairs for projections
NSQ = 4           # sq tiles of 512
NP = FL // 128    # 4 feature chunks (head pairs)
NT16 = S // 128   # 16 seq chunks of 128

F32 = mybir.dt.float32
BF16 = mybir.dt.bfloat16
FP8 = mybir.dt.float8e4
EXP = mybir.ActivationFunctionType.Exp
DR = mybir.MatmulPerfMode.DoubleRow

_CACHED = {}


def _alu():
    from concourse.alu_op_type import AluOpType
    return AluOpType


def _emit_body(nc, P, dram, rep):
    """One full forward pass for this core's shard."""
    mult = _alu().mult
    is_ge = _alu().is_ge
    d_x8, d_xb, d_wq, d_wk, d_wv, d_wo, d_sin, d_out = dram
    (p_x8, p_xb, p_w, p_wv, p_wo, p_qk, p_v, p_sin, p_pt, p_yt, p_r, p_os,
     ps_proj, ps_s, ps_o) = P
    R = f"r{rep}"

    # ---- resident loads: Q/K operands first so PE can start early ----
    x8_sb, xb_sb = [], []
    for j in range(NKP):
        t = p_x8.tile([128, 2, S], FP8, tag="x8", name=f"{R}x8_{j}")
        nc.sync.dma_start(t[:], d_x8[j])
        x8_sb.append(t)
    import os as _os0
    _wdt = BF16 if _os0.environ.get("QK_BF16") else FP8
    wq_sb, wk_sb = [], []
    for lst, drm, nm in ((wq_sb, d_wq, "wq"), (wk_sb, d_wk, "wk")):
        for j in range(NKP):
            t = p_w.tile([128, 2, FL], _wdt, tag="w8", name=f"{R}{nm}{j}")
            nc.sync.dma_start(t[:], drm[j])
            lst.append(t)
    sin_sb = p_sin.tile([128, S], F32, tag="sin", name=f"{R}sin_sb")
    nc.sync.dma_start(sin_sb[:], d_sin)
    wqb_sb = (wq_sb, wk_sb)
    wv_sb = []
    for j in range(NKP):
        t = p_wv.tile([128, 2, FL], BF16, tag="wv", name=f"{R}wv{j}")
        nc.sync.dma_start(t[:], d_wv[j])
        wv_sb.append(t)
    for j in range(NKP):
        t = p_xb.tile([128, 2, S], BF16, tag="xb", name=f"{R}xb_{j}")
        nc.sync.dma_start(t[:, :, 0:S // 2], d_xb[j][:, :, 0:S // 2])
        xb_sb.append(t)
    for j in range(NKP):
        nc.sync.dma_start(xb_sb[j][:, :, S // 2:S], d_xb[j][:, :, S // 2:S])
    wo_sb = []
    for p in range(NP):
        t = p_wo.tile([128, D], BF16, tag="wo", name=f"{R}wo{p}")
        nc.sync.dma_start(t[:], d_wo[128 * p:128 * (p + 1), :])
        wo_sb.append(t)

    # ---- V tiles (bf16): chunk-pair tiles [128, 2, HL, 65] ----
    v_sb = []
    for ks in range(NT16 // 2):
        vt = p_v.tile([128, 2, HL, 65], BF16, tag="v", name=f"{R}v{ks}")
        nc.gpsimd.memset(vt[:, :, :, 64:65], 1.0)
        v_sb.append(vt)

    yt_sb = [p_yt.tile([128, S], BF16, tag="yt", name=f"{R}yt{i}")
             for i in range(NP)]
    qt_sb = [p_qk.tile([128, S], BF16, tag="qk", name=f"{R}qt{i}")
             for i in range(NP)]
    kt_sb = [p_qk.tile([128, S], BF16, tag="qk", name=f"{R}kt{i}")
             for i in range(NP)]

    # The Tile scheduler keeps each engine close to emission order, so
    # projection work is emitted as single-psum-group "filler" units woven
    # into the attention chunk loops: PE stays busy while ACT grinds exp,
    # and the softmax epilogue latency is hidden behind independent matmuls.
    from collections import deque
    filler = deque()

    import os as _os
    qk_bf16 = bool(_os.environ.get("QK_BF16"))

    def emit_qk(p, t, which):
        # Q (which=0) / K (which=1) projection: fp8 DoubleRow, rope (sin
        # mult) fused into the PSUM drain
        w_sb = (wq_sb, wk_sb)[which]
        dst = (qt_sb, kt_sb)[which][p]
        ps = ps_proj.tile([128, 512], F32, tag="psp",
                          name=f"{R}qkps{which}{p}{t}")
        if qk_bf16:
            for j in range(NKP):
                for i in range(2):
                    nc.tensor.matmul(
                        ps[:],
                        wqb_sb[which][j][:, i, 128 * p:128 * (p + 1)],
                        xb_sb[j][:, i, 512 * t:512 * (t + 1)],
                        start=(j == 0 and i == 0),
                        stop=(j == NKP - 1 and i == 1))
        else:
            for j in range(NKP):
                nc.tensor.matmul(
                    ps[:],
                    w_sb[j][:, :, 128 * p:128 * (p + 1)],
                    x8_sb[j][:, :, 512 * t:512 * (t + 1)],
                    start=(j == 0), stop=(j == NKP - 1), perf_mode=DR)
        nc.vector.tensor_tensor(
            out=dst[:, 512 * t:512 * (t + 1)],
            in0=ps[:], in1=sin_sb[:, 512 * t:512 * (t + 1)],
            op=mult)

    def emit_v(q):
        # V projection for seq chunk q (bf16)
        ps = ps_proj.tile([128, FL], F32, tag="psp", name=f"{R}vps{q}")
        for j in range(NKP):
            for i in range(2):
                nc.tensor.matmul(
                    ps[:], xb_sb[j][:, i, 128 * q:128 * (q + 1)],
                    wv_sb[j][:, i, :],
                    start=(j == 0 and i == 0),
                    stop=(j == NKP - 1 and i == 1))
        nc.vector.tensor_copy(
            v_sb[q // 2][:, q % 2, :, 0:64],
            ps[:].rearrange("p (h d) -> p h d", h=HL))

    def emit_o(q, o):
        # output projection for seq chunk q, half o: [sq, outD] layout
        ps = ps_proj.tile([128, 512], F32, tag="psp", name=f"{R}ops{q}{o}")
        for p in range(NP):
            nc.tensor.matmul(
                ps[:],
                yt_sb[p][:, 128 * q:128 * (q + 1)],
                wo_sb[p][:, 512 * o:512 * (o + 1)],
                start=(p == 0), stop=(p == NP - 1))
        os_t = p_os.tile([128, 512], F32, tag="os", name=f"{R}os{q}{o}")
        nc.vector.tensor_copy(os_t[:], ps[:])
        nc.sync.dma_start(
            d_out[128 * q:128 * (q + 1), 512 * o:512 * (o + 1)], os_t[:])

    def pop_filler():
        if filler:
            filler.popleft()[1]()

    def force(keys):
        # emit queued units matching `keys` now (dependencies of the next
        # attention block), preserving queue order of the rest
        keys = set(keys)
        kept = []
        for key, fn in filler:
            if key in keys:
                fn()
            else:
                kept.append((key, fn))
        filler.clear()
        filler.extend(kept)

    # preamble: pair 0 / tile 0 operands emitted directly
    emit_qk(0, 0, 0)
    emit_qk(0, 0, 1)
    for q in range(4):
        emit_v(q)
    for p in range(1, NP):
        for w in range(2):
            filler.append((("qk", p, 0, w), lambda p=p, w=w: emit_qk(p, 0, w)))

    for t in range(NSQ):
        # stage filler: all remaining QK projections go out during tile 0 so
        # x8/wq/wk/sin free early (next rep's DMAs prefetch behind compute);
        # V projections stay one tile ahead; O-proj fills the most
        # ACT-bound tile (t=3)
        if t == 0:
            for tn in range(1, NSQ):
                for p in range(NP):
                    for w in range(2):
                        filler.append((("qk", p, tn, w),
                                       lambda p=p, w=w, tn=tn:
                                       emit_qk(p, tn, w)))
        if t + 1 < NSQ:
            for q in range(4 * (t + 1), 4 * (t + 2)):
                filler.append((("v", q), lambda q=q: emit_v(q)))
        if t == NSQ - 1:
            for q in range(4 * (NSQ - 1)):
                for o in range(2):
                    filler.append((("o", q, o),
                                   lambda q=q, o=o: emit_o(q, o)))

        for p in range(NP):
            # dependencies of this attention block must be emitted first
            need = [("qk", p, t, w) for w in range(2)]
            need += [("v", q) for q in range(4 * (t + 1))]
            force(need)

            # attention for heads 2p (e=0) and 2p+1 (e=1). Scores for one
            # 128-k-chunk land in a [128, 2(e), 512] psum written at TRUE
            # columns (diagonal chunks restricted to [off:512]); exp and
            # causal affine_select window the same range; P@V runs one
            # chunk behind the scores.
            nchunks = 4 * (t + 1)
            o_ps = [ps_o.tile([65, 512], F32, tag="pso",
                              name=f"{R}o{p}_{t}_{i}") for i in range(2)]
            pv_prev = None
            for c in range(nchunks):
                cc = c - 4 * t
                off = 128 * cc if cc > 0 else 0
                s_ps = ps_s.tile([128, 2, 512], F32, tag="s",
                                 name=f"{R}s{p}_{t}_{c}")
                for e in range(2):
                    nc.tensor.matmul(
                        s_ps[:, e, off:512],
                        kt_sb[p][64 * e:64 * (e + 1),
                                 128 * c:128 * (c + 1)],
                        qt_sb[p][64 * e:64 * (e + 1),
                                 512 * t + off:512 * (t + 1)],
                        start=True, stop=True)
                pt = p_pt.tile([128, 2, 512], BF16, tag="pt",
                               name=f"{R}pt{p}_{t}_{c}")
                nc.scalar.activation(
                    pt[:, :, off:512], s_ps[:, :, off:512],
                    EXP, scale=1.0 / 512.0)
                if cc >= 0:
                    # causal: keep where (off + q) - part - 128*cc >= 0
                    nc.gpsimd.affine_select(
                        out=pt[:, :, off:512],
                        in_=pt[:, :, off:512],
                        compare_op=is_ge,
                        fill=0.0,
                        base=off - 128 * cc,
                        pattern=[[0, 2], [1, 512 - off]],
                        channel_multiplier=-1)
                if pv_prev is not None:
                    cp, ptp, offp = pv_prev
                    for e in range(2):
                        nc.tensor.matmul(
                            o_ps[e][:, offp:512],
                            v_sb[cp // 2][:, cp % 2, 2 * p + e, 0:65],
                            ptp[:, e, offp:512],
                            start=(cp == 0), stop=False)
                pv_prev = (c, pt, off)
                pop_filler()
            cp, ptp, offp = pv_prev
            for e in range(2):
                nc.tensor.matmul(
                    o_ps[e][:, offp:512],
                    v_sb[cp // 2][:, cp % 2, 2 * p + e, 0:65],
                    ptp[:, e, offp:512],
                    start=(cp == 0), stop=True)
            for e in range(2):
                recip = p_r.tile([1, 512], F32, tag="rc",
                                 name=f"{R}rc{p}_{t}{e}")
                nc.vector.reciprocal(recip[:], o_ps[e][64:65, :])
                rb = p_r.tile([64, 512], F32, tag="rb",
                              name=f"{R}rb{p}_{t}{e}")
                nc.gpsimd.partition_broadcast(rb[:], recip[:], channels=64)
                nc.vector.tensor_tensor(
                    out=yt_sb[p][64 * e:64 * (e + 1),
                                 512 * t:512 * (t + 1)],
                    in0=o_ps[e][0:64, :], in1=rb[:], op=mult)

    # drain leftovers + last tile's output projection
    while filler:
        pop_filler()
    for q in range(4 * (NSQ - 1), 4 * NSQ):
        for o in range(2):
            emit_o(q, o)


def build_nc(reps=1):
    key = ("nc", reps)
    if key in _CACHED:
        return _CACHED[key]
    from contextlib import ExitStack

    import os
    nc = bacc.Bacc("TRN2", target_bir_lowering=False, debug=False,
                   num_devices=NCORES)
    dram = (
        nc.dram_tensor("x8", [NKP, 128, 2, S], FP8, kind="ExternalInput").ap(),
        nc.dram_tensor("xb", [NKP, 128, 2, S], BF16, kind="ExternalInput").ap(),
        nc.dram_tensor("wq", [NKP, 128, 2, FL],
                       BF16 if os.environ.get("QK_BF16") else FP8,
                       kind="ExternalInput").ap(),
        nc.dram_tensor("wk", [NKP, 128, 2, FL],
                       BF16 if os.environ.get("QK_BF16") else FP8,
                       kind="ExternalInput").ap(),
        nc.dram_tensor("wv", [NKP, 128, 2, FL], BF16, kind="ExternalInput").ap(),
        nc.dram_tensor("wo", [FL, D], BF16, kind="ExternalInput").ap(),
        nc.dram_tensor("sin", [128, S], F32, kind="ExternalInput").ap(),
        nc.dram_tensor("out", [S, D], F32, kind="ExternalOutput").ap(),
    )

    import os
    trace_sim = bool(os.environ.get("KTRACE"))
    with tile.TileContext(nc, trace_sim=trace_sim) as tc, ExitStack() as ctx:
        P = (
            ctx.enter_context(tc.tile_pool(name="x8", bufs=NKP)),
            ctx.enter_context(tc.tile_pool(name="xb", bufs=(NKP if os.environ.get("QK_BF16") else 2 * NKP))),
            ctx.enter_context(tc.tile_pool(name="w", bufs=2 * NKP)),
            ctx.enter_context(tc.tile_pool(name="wv", bufs=2 * NKP)),
            ctx.enter_context(tc.tile_pool(name="wo", bufs=NP)),
            ctx.enter_context(tc.tile_pool(name="qk", bufs=2 * NP)),
            ctx.enter_context(tc.tile_pool(name="v", bufs=NT16 // 2)),
            ctx.enter_context(tc.tile_pool(name="sin", bufs=1)),
            ctx.enter_context(tc.tile_pool(name="pt", bufs=3)),
            ctx.enter_context(tc.tile_pool(name="yt", bufs=NP)),
            ctx.enter_context(tc.tile_pool(name="r", bufs=2)),
            ctx.enter_context(tc.tile_pool(name="os", bufs=2)),
            ctx.enter_context(tc.tile_pool(name="psp", bufs=2, space="PSUM")),
            ctx.enter_context(tc.tile_pool(name="pss", bufs=2, space="PSUM")),
            ctx.enter_context(tc.tile_pool(name="pso", bufs=2, space="PSUM")),
        )
        for rep in range(reps):
            _emit_body(nc, P, dram, rep)

    nc.finalize()
    _CACHED[key] = nc
    return nc


def _host_prep(x, wq, wk, wv, wo):
    """Fold RoPE rotation into wq/wk; scale for fp8; slice per core."""
    import os
    # sin table exactly as the reference computes it (f32 throughout)
    rope_dim = AOD // 2
    j = np.arange(rope_dim, dtype=np.float32)
    thetas = (1.0 / (10000.0 ** (2.0 * j / rope_dim))).astype(np.float32)
    pos = np.arange(S, dtype=np.float32)
    angles = pos[:, None] * thetas[None, :]          # [S, 32]
    sinv = np.sin(angles).astype(np.float32)         # [S, 32]
    # sin pattern tile [128, S]: row r multiplies feature (64*pair + r%64);
    # rows r and r+32 (within each head) share sin[:, r%32]
    sin2 = np.tile(sinv.T, (4, 1)).astype(np.float32)  # [128, S]

    def fold(w):
        wr = w.reshape(D, H, 2, rope_dim)
        return np.concatenate(
            [wr[:, :, 0] - wr[:, :, 1], wr[:, :, 0] + wr[:, :, 1]],
            axis=2).reshape(D, D)

    wqf = fold(wq) * 8.0
    wkf = fold(wk) * 8.0

    f8 = ml_dtypes.float8_e4m3
    bf = ml_dtypes.bfloat16

    def pair_pack(a):
        # [D, N] -> [NKP, 128, 2, N] with chunk-pair j = rows 256j..256j+255
        n = a.shape[1]
        return np.ascontiguousarray(
            a.reshape(NKP, 2, 128, n).transpose(0, 2, 1, 3))

    in_maps = []
    for c in range(NCORES):
        b, g = divmod(c, 2)
        sl = slice(g * FL, (g + 1) * FL)
        xp = pair_pack(np.ascontiguousarray(x[b].T))
        in_maps.append({
            "x8": xp.astype(f8),
            "xb": xp.astype(bf),
            "wq": pair_pack(wqf[:, sl]).astype(
                bf if os.environ.get("QK_BF16") else f8),
            "wk": pair_pack(wkf[:, sl]).astype(
                bf if os.environ.get("QK_BF16") else f8),
            "wv": pair_pack(wv[:, sl]).astype(bf),
            "wo": np.ascontiguousarray(wo[sl, :]).astype(bf),
            "sin": sin2,
        })
    return in_maps


def kernel(x, wq, wk, wv, wo, bo):
    nc = build_nc()
    in_maps = _host_prep(np.asarray(x, np.float32), np.asarray(wq, np.float32),
                         np.asarray(wk, np.float32), np.asarray(wv, np.float32),
                         np.asarray(wo, np.float32))
    res = run_bass_kernel_spmd(nc, in_maps, list(range(NCORES)))
    out = np.empty((B, S, D), np.float32)
    bo32 = np.asarray(bo, np.float32)
    for b in range(B):
        out[b] = res.results[2 * b]["out"] + res.results[2 * b + 1]["out"] + bo32
    return out
